# revision 3
# baseline (speedup 1.0000x reference)
"""Trainium2 Bass kernel for nn_EncoderLayer_30425548324932 (hypergraph GNN layer).

Reference computation:
    h       = emb_V @ W_v2e + b_v2e                         [NV, D]
    e_tmp   = relu(segment_mean(h[src], dst, NE))           [NE, D]
    emb_E'  = concat([emb_E, e_tmp], -1) @ W_fuse + b_fuse  [NE, D]
    g       = emb_E' @ W_e2v + b_e2v                        [NE, D]
    emb_V'  = relu(segment_mean(g[dst], src, NV))           [NV, D]
    returns (emb_V', emb_E')

Key algebraic transform: linear layers commute with segment_mean (exact for
non-empty segments; empty segments only differ via the bias, which is zero
here), so we aggregate RAW emb_V rows first and apply W_v2e at segment level:
    e_tmp = relu(segment_mean(emb_V[src], dst, NE) @ W_v2e + b_v2e)
This removes the [NV,D]x[D,D] matmul and the h materialization entirely.

Sharding (8 cores):
  hop1: hyperedges (dst) sharded; each core owns NE/8=2500 hyperedges and the
        edges pointing at them. Gathered emb_V rows come from a per-core
        compacted "ghost node" table (unique srcs, <=32767 rows -> int16 idx).
  hop2: nodes (src) sharded; g (transformed hyperedge features) is AllGathered
        (bf16 hi/lo, 20000x512 = 20.5MB) and gathered per-edge from DRAM.

Precision: gathers/aggregation use bf16 hi+lo split (x = hi + lo, both bf16),
accumulated in fp32 PSUM -> ~1e-5 relative error. Small transform matmuls run
in native fp32.

Segment-sum on device: edges sorted by destination, grouped into windows of
128 segments; per 128-edge block an indicator matrix ind[e,s] =
(dst_local[e]==s) is built with one DVE is_equal and the segment sum is
ind.T @ gathered_rows accumulated in PSUM across the window's blocks.
"""

import numpy as np
import ml_dtypes

import concourse.bass as bass
import concourse.mybir as mybir
import concourse.tile as tile
from concourse import bacc
from concourse.bass_utils import run_bass_kernel_spmd

# Problem shapes (hardcoded per contract).
NV, NE, NEDGE, D = 100000, 20000, 300000, 256
NCORES = 8
NE_SH = NE // NCORES    # 2500 hyperedges per core
NV_SH = NV // NCORES    # 12500 nodes per core
NW1 = (NE_SH + 127) // 128   # 20 hop1 windows (last has 68 segs)
NW2 = (NV_SH + 127) // 128   # 98 hop2 windows (last has 84 nodes)
W1COLS = NW1 * 128           # padded hop1 segment count (2560)
LOCAL_TBL = 32768            # per-core ghost-node table rows
SW2 = 7                      # hop2 windows per dma_gather call (98 = 14*7)
F32 = mybir.dt.float32
BF16 = mybir.dt.bfloat16
I16 = mybir.dt.int16
BF = ml_dtypes.bfloat16


def _hilo(x32):
    """Split fp32 matrix into bf16 hi + bf16 lo with x ~= hi + lo."""
    hi = x32.astype(BF)
    lo = (x32 - hi.astype(np.float32)).astype(BF)
    return hi, lo


def _pack_idx(idx_i16):
    """Pack a flat int16 index list (len % 128 == 0) into the dma_gather SBUF
    layout: index i at [i % 16, i // 16], replicated across the 8 Q7 cores'
    partition groups -> [128, len/16]."""
    return np.tile(idx_i16.reshape(-1, 16).T, (8, 1))


def _shard_edges(sort_key_local, other_end, n_windows, nb):
    """Group this core's edges into fixed-capacity per-window slabs.

    sort_key_local: local destination-segment id per edge (0..n_windows*128).
    other_end: gather index per edge.
    Returns (gidx [n_windows, nb*128] int16, dloc [n_windows, nb*128] f32
    with in-window segment id or -1 for padding).
    """
    order = np.argsort(sort_key_local, kind="stable")
    k = sort_key_local[order]
    idx = other_end[order]
    gidx = np.zeros((n_windows, nb * 128), np.int16)
    dloc = np.full((n_windows, nb * 128), -1.0, np.float32)
    win = k // 128
    starts = np.searchsorted(win, np.arange(n_windows))
    ends = np.searchsorted(win, np.arange(n_windows) + 1)
    for w in range(n_windows):
        s, e = starts[w], ends[w]
        cnt = e - s
        assert cnt <= nb * 128, f"window {w}: {cnt} edges > capacity {nb * 128}"
        gidx[w, :cnt] = idx[s:e]
        dloc[w, :cnt] = (k[s:e] - w * 128).astype(np.float32)
    return gidx, dloc


def build_kernel(nb1, nb2):
    nc = bacc.Bacc("TRN2", target_bir_lowering=False, debug=False,
                   num_devices=NCORES)

    # ---- I/O ----
    emb_loc = nc.dram_tensor("emb_loc", [LOCAL_TBL, 2 * D], BF16, kind="ExternalInput")
    embET = nc.dram_tensor("embET", [2 * 128, W1COLS], F32, kind="ExternalInput")
    wv2e = nc.dram_tensor("wv2e", [D, D], F32, kind="ExternalInput")
    wfuse = nc.dram_tensor("wfuse", [2 * D, D], F32, kind="ExternalInput")
    we2v = nc.dram_tensor("we2v", [D, D], F32, kind="ExternalInput")
    bv2e = nc.dram_tensor("bv2e", [128, 2], F32, kind="ExternalInput")
    bfuse = nc.dram_tensor("bfuse", [128, 2], F32, kind="ExternalInput")
    be2v = nc.dram_tensor("be2v", [128, D], F32, kind="ExternalInput")
    iota_in = nc.dram_tensor("iota_in", [128, 128], BF16, kind="ExternalInput")
    ident_in = nc.dram_tensor("ident_in", [128, 128], F32, kind="ExternalInput")
    idx1 = nc.dram_tensor("idx1", [128, NW1 * nb1 * 8], I16, kind="ExternalInput")
    dstloc1 = nc.dram_tensor("dstloc1", [128, NW1 * nb1], BF16, kind="ExternalInput")
    invc1 = nc.dram_tensor("invc1", [128, NW1], F32, kind="ExternalInput")
    idx2 = nc.dram_tensor("idx2", [128, NW2 * nb2 * 8], I16, kind="ExternalInput")
    dstloc2 = nc.dram_tensor("dstloc2", [128, NW2 * nb2], BF16, kind="ExternalInput")
    invc2 = nc.dram_tensor("invc2", [128, NW2], F32, kind="ExternalInput")

    embE_new = nc.dram_tensor("embE_new", [NE_SH, D], F32, kind="ExternalOutput")
    embV_new = nc.dram_tensor("embV_new", [NV_SH, D], F32, kind="ExternalOutput")

    with tile.TileContext(nc, num_cores=NCORES) as tc:
        with (
            tc.tile_pool(name="const", bufs=1) as cp,
            tc.tile_pool(name="g1p", bufs=2) as g1p,
            tc.tile_pool(name="g2p", bufs=2) as g2p,
            tc.tile_pool(name="indp", bufs=2) as indp,
            tc.tile_pool(name="work", bufs=2) as wp,
            tc.tile_pool(name="psb", bufs=3, space="PSUM") as psb,
            tc.tile_pool(name="pss", bufs=4, space="PSUM") as pss,
            tc.tile_pool(name="dram", bufs=1, space="DRAM") as dr,
        ):
            # ---- load constants ----
            def load(name, src, shape, dtype):
                t = cp.tile(shape, dtype, name=name)
                nc.sync.dma_start(t[:], src)
                return t

            def load_chunked(name, src, nchunks, width, dtype):
                # [nchunks*128, width] DRAM -> SBUF [128, nchunks*width],
                # chunk c at cols [c*width, (c+1)*width)
                t = cp.tile([128, nchunks * width], dtype, name=name)
                for c in range(nchunks):
                    nc.sync.dma_start(t[:, c * width:(c + 1) * width],
                                      src[c * 128:(c + 1) * 128, :])
                return t

            embET_sb = load_chunked("embET_sb", embET, 2, W1COLS, F32)
            wv2e_sb = load_chunked("wv2e_sb", wv2e, 2, D, F32)
            wfuse_sb = load_chunked("wfuse_sb", wfuse, 4, D, F32)
            we2v_sb = load_chunked("we2v_sb", we2v, 2, D, F32)
            bv2e_sb = load("bv2e_sb", bv2e[:, :], [128, 2], F32)
            bfuse_sb = load("bfuse_sb", bfuse[:, :], [128, 2], F32)
            be2v_sb = load("be2v_sb", be2v[:, :], [128, D], F32)
            iota_sb = load("iota_sb", iota_in[:, :], [128, 128], BF16)
            ident_sb = load("ident_sb", ident_in[:, :], [128, 128], F32)
            idx1_sb = load("idx1_sb", idx1[:, :], [128, NW1 * nb1 * 8], I16)
            dstloc1_sb = load("dstloc1_sb", dstloc1[:, :], [128, NW1 * nb1], BF16)
            invc1_sb = load("invc1_sb", invc1[:, :], [128, NW1], F32)
            idx2_sb = load("idx2_sb", idx2[:, :], [128, NW2 * nb2 * 8], I16)
            dstloc2_sb = load("dstloc2_sb", dstloc2[:, :], [128, NW2 * nb2], BF16)
            invc2_sb = load("invc2_sb", invc2[:, :], [128, NW2], F32)

            g_loc = dr.tile([NE_SH, 2 * D], BF16)
            g_full = dr.tile([NE, 2 * D], BF16, addr_space="Shared")

            # ================= HOP 1 =================
            for w in range(NW1):
                nseg = NE_SH - w * 128 if w == NW1 - 1 else 128
                nidx = nb1 * 128

                ghl = g1p.tile([128, nb1, 2 * D], BF16, name="ghl")
                nc.gpsimd.dma_gather(
                    out_ap=ghl[:],
                    in_ap=emb_loc[:, :],
                    idxs_ap=idx1_sb[:, w * nb1 * 8:(w + 1) * nb1 * 8],
                    num_idxs=nidx,
                    num_idxs_reg=nidx,
                    elem_size=2 * D,
                    single_packet=False,
                )

                ind = indp.tile([128, nb1, 128], BF16, name="ind", tag="ind")
                nc.vector.tensor_tensor(
                    out=ind[:],
                    in0=dstloc1_sb[:, w * nb1:(w + 1) * nb1]
                        .unsqueeze(2).broadcast_to([128, nb1, 128]),
                    in1=iota_sb[:].unsqueeze(1).broadcast_to([128, nb1, 128]),
                    op=mybir.AluOpType.is_equal,
                )

                ps_agg = psb.tile([128, D], F32, name="ps_agg", tag="psb")
                n_mm = 2 * nb1
                mm = 0
                for b in range(nb1):
                    for half in range(2):
                        nc.tensor.matmul(
                            out=ps_agg[:],
                            lhsT=ind[:, b, :],
                            rhs=ghl[:, b, half * D:(half + 1) * D],
                            start=(mm == 0),
                            stop=(mm == n_mm - 1),
                        )
                        mm += 1

                # e_mean = psum * (1/cnt)  [seg, d]
                e_mean = wp.tile([128, D], F32, name="e_mean")
                nc.scalar.activation(
                    out=e_mean[:], in_=ps_agg[:],
                    func=mybir.ActivationFunctionType.Copy,
                    scale=invc1_sb[:, w:w + 1],
                )

                # transpose -> e_meanT [d, seg] (2 chunks of 128)
                e_meanT = wp.tile([128, D], F32, name="e_meanT")
                for c in range(2):
                    pt = pss.tile([128, 128], F32, name="pt_mean", tag="pss")
                    nc.tensor.transpose(pt[:], e_mean[:, c * 128:(c + 1) * 128],
                                        ident_sb[:])
                    nc.scalar.copy(e_meanT[:, c * 128:(c + 1) * 128], pt[:])

                # e_tmpT = relu(W_v2e-transform + b)  [d', seg]
                e_tmpT = wp.tile([128, D], F32, name="e_tmpT")
                for m in range(2):
                    pv = pss.tile([128, 128], F32, name="pv", tag="pss")
                    for kc in range(2):
                        nc.tensor.matmul(
                            out=pv[:],
                            lhsT=wv2e_sb[:, kc * D + m * 128:kc * D + (m + 1) * 128],
                            rhs=e_meanT[:, kc * 128:(kc + 1) * 128],
                            start=(kc == 0), stop=(kc == 1),
                        )
                    nc.scalar.activation(
                        out=e_tmpT[:, m * 128:(m + 1) * 128], in_=pv[:],
                        func=mybir.ActivationFunctionType.Relu,
                        bias=bv2e_sb[:, m:m + 1],
                    )

                # embE_newT = W_fuse-transform([embET; e_tmpT]) + b_fuse  [d', seg]
                enT = wp.tile([128, D], F32, name="enT")
                for m in range(2):
                    pf = pss.tile([128, 128], F32, name="pf", tag="pss")
                    for kc in range(4):
                        if kc < 2:
                            rhs = embET_sb[:, kc * W1COLS + w * 128:
                                           kc * W1COLS + (w + 1) * 128]
                        else:
                            rhs = e_tmpT[:, (kc - 2) * 128:(kc - 1) * 128]
                        nc.tensor.matmul(
                            out=pf[:],
                            lhsT=wfuse_sb[:, kc * D + m * 128:kc * D + (m + 1) * 128],
                            rhs=rhs,
                            start=(kc == 0), stop=(kc == 3),
                        )
                    nc.scalar.activation(
                        out=enT[:, m * 128:(m + 1) * 128], in_=pf[:],
                        func=mybir.ActivationFunctionType.Identity,
                        bias=bfuse_sb[:, m:m + 1],
                    )

                # g = embE_new @ W_e2v (+ b_e2v)  [seg, d']  (row-major)
                ps_g = psb.tile([128, D], F32, name="ps_g", tag="psb")
                for kc in range(2):
                    nc.tensor.matmul(
                        out=ps_g[:],
                        lhsT=enT[:, kc * 128:(kc + 1) * 128],
                        rhs=we2v_sb[:, kc * D:(kc + 1) * D],
                        start=(kc == 0), stop=(kc == 1),
                    )
                gf = wp.tile([128, D], F32, name="gf")
                nc.vector.tensor_tensor(out=gf[:], in0=ps_g[:], in1=be2v_sb[:],
                                        op=mybir.AluOpType.add)
                # bf16 hi/lo split, packed [hi | lo]
                ghilo = wp.tile([128, 2 * D], BF16, name="ghilo")
                nc.scalar.copy(ghilo[:, :D], gf[:])
                ghi32 = wp.tile([128, D], F32, name="ghi32")
                nc.scalar.copy(ghi32[:], ghilo[:, :D])
                nc.vector.tensor_tensor(out=ghilo[:, D:], in0=gf[:], in1=ghi32[:],
                                        op=mybir.AluOpType.subtract)
                nc.sync.dma_start(g_loc[w * 128:w * 128 + nseg, :], ghilo[:nseg, :])

                # emb_E_new output (un-transpose enT)
                en = wp.tile([128, D], F32, name="en")
                for c in range(2):
                    pt2 = pss.tile([128, 128], F32, name="pt_en", tag="pss")
                    nc.tensor.transpose(pt2[:], enT[:, c * 128:(c + 1) * 128],
                                        ident_sb[:])
                    nc.scalar.copy(en[:, c * 128:(c + 1) * 128], pt2[:])
                nc.sync.dma_start(embE_new[w * 128:w * 128 + nseg, :], en[:nseg, :])

            # ================= AllGather g =================
            nc.gpsimd.collective_compute(
                "AllGather",
                mybir.AluOpType.bypass,
                replica_groups=[list(range(NCORES))],
                ins=[g_loc.opt()],
                outs=[g_full.opt()],
            )

            # ================= HOP 2 =================
            for call in range(NW2 // SW2):
                nidx = SW2 * nb2 * 128
                ghl2 = g2p.tile([128, SW2 * nb2, 2 * D], BF16, name="ghl2")
                nc.gpsimd.dma_gather(
                    out_ap=ghl2[:],
                    in_ap=g_full[:, :],
                    idxs_ap=idx2_sb[:, call * SW2 * nb2 * 8:
                                    (call + 1) * SW2 * nb2 * 8],
                    num_idxs=nidx,
                    num_idxs_reg=nidx,
                    elem_size=2 * D,
                    single_packet=False,
                )
                for j in range(SW2):
                    w = call * SW2 + j
                    nnode = NV_SH - w * 128 if w == NW2 - 1 else 128

                    ind2 = indp.tile([128, nb2, 128], BF16, name="ind2", tag="ind")
                    nc.vector.tensor_tensor(
                        out=ind2[:],
                        in0=dstloc2_sb[:, w * nb2:(w + 1) * nb2]
                            .unsqueeze(2).broadcast_to([128, nb2, 128]),
                        in1=iota_sb[:].unsqueeze(1).broadcast_to([128, nb2, 128]),
                        op=mybir.AluOpType.is_equal,
                    )

                    ps2 = psb.tile([128, D], F32, name="ps2", tag="psb")
                    n_mm = 2 * nb2
                    mm = 0
                    for b in range(nb2):
                        for half in range(2):
                            nc.tensor.matmul(
                                out=ps2[:],
                                lhsT=ind2[:, b, :],
                                rhs=ghl2[:, j * nb2 + b, half * D:(half + 1) * D],
                                start=(mm == 0),
                                stop=(mm == n_mm - 1),
                            )
                            mm += 1

                    vout = wp.tile([128, D], F32, name="vout")
                    nc.scalar.activation(
                        out=vout[:], in_=ps2[:],
                        func=mybir.ActivationFunctionType.Relu,
                        scale=invc2_sb[:, w:w + 1],
                    )
                    nc.sync.dma_start(embV_new[w * 128:w * 128 + nnode, :],
                                      vout[:nnode, :])

    nc.compile()
    return nc


_CACHE = {}


def kernel(emb_V, emb_E, edge_index, W_v2e, b_v2e, W_fuse, b_fuse, W_e2v, b_e2v):
    emb_V = np.asarray(emb_V, np.float32)
    emb_E = np.asarray(emb_E, np.float32)
    edge_index = np.asarray(edge_index)
    src = edge_index[0].astype(np.int64)
    dst = edge_index[1].astype(np.int64)

    inv_e = 1.0 / np.maximum(np.bincount(dst, minlength=NE), 1.0).astype(np.float32)
    inv_v = 1.0 / np.maximum(np.bincount(src, minlength=NV), 1.0).astype(np.float32)

    # ---- per-core graph partitioning; uniform block counts ----
    core_of_dst = dst // NE_SH
    core_of_src = src // NV_SH
    nb1 = nb2 = 1
    pre1, pre2 = [], []
    for k in range(NCORES):
        m1 = core_of_dst == k
        s1, d1 = src[m1], dst[m1] - k * NE_SH
        u, linv = np.unique(s1, return_inverse=True)
        assert len(u) <= LOCAL_TBL, f"core {k}: {len(u)} unique srcs"
        pre1.append((u, linv.astype(np.int64), d1))
        c1 = np.bincount(d1 // 128, minlength=NW1)
        nb1 = max(nb1, int(np.ceil(c1.max() / 128)))

        m2 = core_of_src == k
        s2, d2 = src[m2] - k * NV_SH, dst[m2]
        pre2.append((s2, d2))
        c2 = np.bincount(s2 // 128, minlength=NW2)
        nb2 = max(nb2, int(np.ceil(c2.max() / 128)))

    key = (nb1, nb2)
    if key not in _CACHE:
        _CACHE[key] = build_kernel(nb1, nb2)
    nc = _CACHE[key]

    # ---- shared constant inputs ----
    iota = np.broadcast_to(np.arange(128, dtype=np.float32), (128, 128)).astype(BF)
    ident = np.eye(128, dtype=np.float32)
    bv2e_t = np.asarray(b_v2e, np.float32).reshape(2, 128).T.copy()
    bfuse_t = np.asarray(b_fuse, np.float32).reshape(2, 128).T.copy()
    be2v_b = np.broadcast_to(np.asarray(b_e2v, np.float32), (128, D)).copy()

    in_maps = []
    for k in range(NCORES):
        u, linv, d1 = pre1[k]
        gidx1, dloc1 = _shard_edges(d1, linv, NW1, nb1)
        s2, d2 = pre2[k]
        gidx2, dloc2 = _shard_edges(s2, d2, NW2, nb2)

        hi, lo = _hilo(emb_V[u])
        emb_loc = np.zeros((LOCAL_TBL, 2 * D), BF)
        emb_loc[:len(u), :D] = hi
        emb_loc[:len(u), D:] = lo

        embET = np.zeros((2 * 128, W1COLS), np.float32)
        embET[:, :NE_SH] = emb_E[k * NE_SH:(k + 1) * NE_SH].T

        idx1_arr = np.concatenate(
            [_pack_idx(gidx1[w]) for w in range(NW1)], axis=1)
        idx2_arr = np.concatenate(
            [_pack_idx(gidx2[c * SW2:(c + 1) * SW2].reshape(-1))
             for c in range(NW2 // SW2)], axis=1)

        dstloc1 = np.ascontiguousarray(dloc1.reshape(NW1 * nb1, 128).T).astype(BF)
        dstloc2 = np.ascontiguousarray(dloc2.reshape(NW2 * nb2, 128).T).astype(BF)

        invc1_full = np.ones(NW1 * 128, np.float32)
        invc1_full[:NE_SH] = inv_e[k * NE_SH:(k + 1) * NE_SH]
        invc1 = np.ascontiguousarray(invc1_full.reshape(NW1, 128).T)

        invc2_full = np.ones(NW2 * 128, np.float32)
        invc2_full[:NV_SH] = inv_v[k * NV_SH:(k + 1) * NV_SH]
        invc2 = np.ascontiguousarray(invc2_full.reshape(NW2, 128).T)

        in_maps.append(dict(
            emb_loc=emb_loc, embET=embET,
            wv2e=np.asarray(W_v2e, np.float32),
            wfuse=np.asarray(W_fuse, np.float32),
            we2v=np.asarray(W_e2v, np.float32),
            bv2e=bv2e_t, bfuse=bfuse_t, be2v=be2v_b,
            iota_in=iota, ident_in=ident,
            idx1=idx1_arr, dstloc1=dstloc1, invc1=invc1,
            idx2=idx2_arr, dstloc2=dstloc2, invc2=invc2,
        ))

    res = run_bass_kernel_spmd(nc, in_maps, core_ids=list(range(NCORES)))
    emb_V_new = np.concatenate([res.results[k]["embV_new"] for k in range(NCORES)])
    emb_E_new = np.concatenate([res.results[k]["embE_new"] for k in range(NCORES)])
    return emb_V_new, emb_E_new


# revision 4
# speedup vs baseline: 8336.0572x; 8336.0572x over previous
"""Trainium2 Bass kernel for nn_EncoderLayer_30425548324932 (hypergraph GNN layer).

Reference computation:
    h       = emb_V @ W_v2e + b_v2e                         [NV, D]
    e_tmp   = relu(segment_mean(h[src], dst, NE))           [NE, D]
    emb_E'  = concat([emb_E, e_tmp], -1) @ W_fuse + b_fuse  [NE, D]
    g       = emb_E' @ W_e2v + b_e2v                        [NE, D]
    emb_V'  = relu(segment_mean(g[dst], src, NV))           [NV, D]
    returns (emb_V', emb_E')

Key algebraic transform: linear layers commute with segment_mean (exact for
non-empty segments; empty segments only differ via the bias, which is zero
here), so we aggregate RAW emb_V rows first and apply W_v2e at segment level:
    e_tmp = relu(segment_mean(emb_V[src], dst, NE) @ W_v2e + b_v2e)
This removes the [NV,D]x[D,D] matmul and the h materialization entirely.

Sharding (8 cores):
  hop1: hyperedges (dst) sharded; each core owns NE/8=2500 hyperedges and the
        edges pointing at them. Gathered emb_V rows come from a per-core
        compacted "ghost node" table (unique srcs, <=32767 rows -> int16 idx).
  hop2: nodes (src) sharded; g (transformed hyperedge features) is AllGathered
        (bf16 hi/lo, 20000x512 = 20.5MB) and gathered per-edge from DRAM.

Precision: gathers/aggregation use bf16 hi+lo split (x = hi + lo, both bf16),
accumulated in fp32 PSUM -> ~1e-5 relative error. Small transform matmuls run
in native fp32.

Segment-sum on device: edges sorted by destination, grouped into windows of
128 segments; per 128-edge block an indicator matrix ind[e,s] =
(dst_local[e]==s) is built with one DVE is_equal and the segment sum is
ind.T @ gathered_rows accumulated in PSUM across the window's blocks.
"""

import numpy as np
import ml_dtypes

import concourse.bass as bass
import concourse.mybir as mybir
import concourse.tile as tile
from concourse import bacc
from concourse.bass_utils import run_bass_kernel_spmd

# Problem shapes (hardcoded per contract).
NV, NE, NEDGE, D = 100000, 20000, 300000, 256
NCORES = 8
NE_SH = NE // NCORES    # 2500 hyperedges per core
NV_SH = NV // NCORES    # 12500 nodes per core
NW1 = (NE_SH + 127) // 128   # 20 hop1 windows (last has 68 segs)
NW2 = (NV_SH + 127) // 128   # 98 hop2 windows (last has 84 nodes)
W1COLS = NW1 * 128           # padded hop1 segment count (2560)
LOCAL_TBL = 32768            # per-core ghost-node table rows
SW2 = 7                      # hop2 windows per dma_gather call (98 = 14*7)
F32 = mybir.dt.float32
BF16 = mybir.dt.bfloat16
I16 = mybir.dt.int16
BF = ml_dtypes.bfloat16


def _hilo(x32):
    """Split fp32 matrix into bf16 hi + bf16 lo with x ~= hi + lo."""
    hi = x32.astype(BF)
    lo = (x32 - hi.astype(np.float32)).astype(BF)
    return hi, lo


def _pack_idx(idx_i16):
    """Pack a flat int16 index list (len % 128 == 0) into the dma_gather SBUF
    layout: index i at [i % 16, i // 16], replicated across the 8 Q7 cores'
    partition groups -> [128, len/16]."""
    return np.tile(idx_i16.reshape(-1, 16).T, (8, 1))


def _shard_edges(sort_key_local, other_end, n_windows, nb):
    """Group this core's edges into fixed-capacity per-window slabs.

    sort_key_local: local destination-segment id per edge (0..n_windows*128).
    other_end: gather index per edge.
    Returns (gidx [n_windows, nb*128] int16, dloc [n_windows, nb*128] f32
    with in-window segment id or -1 for padding).
    """
    order = np.argsort(sort_key_local, kind="stable")
    k = sort_key_local[order]
    idx = other_end[order]
    gidx = np.zeros((n_windows, nb * 128), np.int16)
    dloc = np.full((n_windows, nb * 128), -1.0, np.float32)
    win = k // 128
    starts = np.searchsorted(win, np.arange(n_windows))
    ends = np.searchsorted(win, np.arange(n_windows) + 1)
    for w in range(n_windows):
        s, e = starts[w], ends[w]
        cnt = e - s
        assert cnt <= nb * 128, f"window {w}: {cnt} edges > capacity {nb * 128}"
        gidx[w, :cnt] = idx[s:e]
        dloc[w, :cnt] = (k[s:e] - w * 128).astype(np.float32)
    return gidx, dloc


def build_kernel(nb1, nb2, variant="all"):
    do_hop1 = variant in ("all", "hop1", "hop1_aggonly", "nohop2")
    do_hop1_tail = variant not in ("hop1_aggonly",)
    do_hop2 = variant in ("all", "hop2", "nohop1")
    do_cc = variant == "all"
    nc = bacc.Bacc("TRN2", target_bir_lowering=False, debug=False,
                   num_devices=NCORES)

    # ---- I/O ----
    emb_loc = nc.dram_tensor("emb_loc", [LOCAL_TBL, 2 * D], BF16, kind="ExternalInput")
    embET = nc.dram_tensor("embET", [2 * 128, W1COLS], F32, kind="ExternalInput")
    wv2e = nc.dram_tensor("wv2e", [D, D], F32, kind="ExternalInput")
    wfuse = nc.dram_tensor("wfuse", [2 * D, D], F32, kind="ExternalInput")
    we2v = nc.dram_tensor("we2v", [D, D], F32, kind="ExternalInput")
    bv2e = nc.dram_tensor("bv2e", [128, 2], F32, kind="ExternalInput")
    bfuse = nc.dram_tensor("bfuse", [128, 2], F32, kind="ExternalInput")
    be2v = nc.dram_tensor("be2v", [128, D], F32, kind="ExternalInput")
    iota_in = nc.dram_tensor("iota_in", [128, 128], BF16, kind="ExternalInput")
    ident_in = nc.dram_tensor("ident_in", [128, 128], F32, kind="ExternalInput")
    idx1 = nc.dram_tensor("idx1", [128, NW1 * nb1 * 8], I16, kind="ExternalInput")
    dstloc1 = nc.dram_tensor("dstloc1", [128, NW1 * nb1], BF16, kind="ExternalInput")
    invc1 = nc.dram_tensor("invc1", [128, NW1], F32, kind="ExternalInput")
    idx2 = nc.dram_tensor("idx2", [128, NW2 * nb2 * 8], I16, kind="ExternalInput")
    dstloc2 = nc.dram_tensor("dstloc2", [128, NW2 * nb2], BF16, kind="ExternalInput")
    invc2 = nc.dram_tensor("invc2", [128, NW2], F32, kind="ExternalInput")

    embE_new = nc.dram_tensor("embE_new", [NE_SH, D], F32, kind="ExternalOutput")
    embV_new = nc.dram_tensor("embV_new", [NV_SH, D], F32, kind="ExternalOutput")

    with tile.TileContext(nc, num_cores=NCORES) as tc:
        with (
            tc.tile_pool(name="const", bufs=1) as cp,
            tc.tile_pool(name="g1p", bufs=2) as g1p,
            tc.tile_pool(name="g2p", bufs=2) as g2p,
            tc.tile_pool(name="indp", bufs=2) as indp,
            tc.tile_pool(name="work", bufs=2) as wp,
            tc.tile_pool(name="psb", bufs=3, space="PSUM") as psb,
            tc.tile_pool(name="pss", bufs=4, space="PSUM") as pss,
            tc.tile_pool(name="dram", bufs=1, space="DRAM") as dr,
        ):
            # ---- load constants ----
            def load(name, src, shape, dtype):
                t = cp.tile(shape, dtype, name=name)
                nc.sync.dma_start(t[:], src)
                return t

            def load_chunked(name, src, nchunks, width, dtype):
                # [nchunks*128, width] DRAM -> SBUF [128, nchunks*width],
                # chunk c at cols [c*width, (c+1)*width)
                t = cp.tile([128, nchunks * width], dtype, name=name)
                for c in range(nchunks):
                    nc.sync.dma_start(t[:, c * width:(c + 1) * width],
                                      src[c * 128:(c + 1) * 128, :])
                return t

            embET_sb = load_chunked("embET_sb", embET, 2, W1COLS, F32)
            wv2e_sb = load_chunked("wv2e_sb", wv2e, 2, D, F32)
            wfuse_sb = load_chunked("wfuse_sb", wfuse, 4, D, F32)
            we2v_sb = load_chunked("we2v_sb", we2v, 2, D, F32)
            bv2e_sb = load("bv2e_sb", bv2e[:, :], [128, 2], F32)
            bfuse_sb = load("bfuse_sb", bfuse[:, :], [128, 2], F32)
            be2v_sb = load("be2v_sb", be2v[:, :], [128, D], F32)
            iota_sb = load("iota_sb", iota_in[:, :], [128, 128], BF16)
            ident_sb = load("ident_sb", ident_in[:, :], [128, 128], F32)
            idx1_sb = load("idx1_sb", idx1[:, :], [128, NW1 * nb1 * 8], I16)
            dstloc1_sb = load("dstloc1_sb", dstloc1[:, :], [128, NW1 * nb1], BF16)
            invc1_sb = load("invc1_sb", invc1[:, :], [128, NW1], F32)
            idx2_sb = load("idx2_sb", idx2[:, :], [128, NW2 * nb2 * 8], I16)
            dstloc2_sb = load("dstloc2_sb", dstloc2[:, :], [128, NW2 * nb2], BF16)
            invc2_sb = load("invc2_sb", invc2[:, :], [128, NW2], F32)

            g_loc = dr.tile([NE_SH, 2 * D], BF16)
            g_full = dr.tile([NE, 2 * D], BF16, addr_space="Shared")

            # ================= HOP 1 =================
            for w in range(NW1 if do_hop1 else 0):
                nseg = NE_SH - w * 128 if w == NW1 - 1 else 128
                nidx = nb1 * 128

                ghl = g1p.tile([128, nb1, 2 * D], BF16, name="ghl")
                nc.gpsimd.dma_gather(
                    out_ap=ghl[:],
                    in_ap=emb_loc[:, :],
                    idxs_ap=idx1_sb[:, w * nb1 * 8:(w + 1) * nb1 * 8],
                    num_idxs=nidx,
                    num_idxs_reg=nidx,
                    elem_size=2 * D,
                    single_packet=False,
                )

                ind = indp.tile([128, nb1, 128], BF16, name="ind", tag="ind")
                nc.vector.tensor_tensor(
                    out=ind[:],
                    in0=dstloc1_sb[:, w * nb1:(w + 1) * nb1]
                        .unsqueeze(2).broadcast_to([128, nb1, 128]),
                    in1=iota_sb[:].unsqueeze(1).broadcast_to([128, nb1, 128]),
                    op=mybir.AluOpType.is_equal,
                )

                ps_agg = psb.tile([128, D], F32, name="ps_agg", tag="psb")
                n_mm = 2 * nb1
                mm = 0
                for b in range(nb1):
                    for half in range(2):
                        nc.tensor.matmul(
                            out=ps_agg[:],
                            lhsT=ind[:, b, :],
                            rhs=ghl[:, b, half * D:(half + 1) * D],
                            start=(mm == 0),
                            stop=(mm == n_mm - 1),
                        )
                        mm += 1

                # e_mean = psum * (1/cnt)  [seg, d]
                e_mean = wp.tile([128, D], F32, name="e_mean")
                nc.scalar.activation(
                    out=e_mean[:], in_=ps_agg[:],
                    func=mybir.ActivationFunctionType.Copy,
                    scale=invc1_sb[:, w:w + 1],
                )

                if not do_hop1_tail:
                    nc.sync.dma_start(embE_new[w * 128:w * 128 + nseg, :],
                                      e_mean[:nseg, :])
                    continue
                # transpose -> e_meanT [d, seg] (2 chunks of 128)
                e_meanT = wp.tile([128, D], F32, name="e_meanT")
                for c in range(2):
                    pt = pss.tile([128, 128], F32, name="pt_mean", tag="pss")
                    nc.tensor.transpose(pt[:], e_mean[:, c * 128:(c + 1) * 128],
                                        ident_sb[:])
                    nc.scalar.copy(e_meanT[:, c * 128:(c + 1) * 128], pt[:])

                # e_tmpT = relu(W_v2e-transform + b)  [d', seg]
                e_tmpT = wp.tile([128, D], F32, name="e_tmpT")
                for m in range(2):
                    pv = pss.tile([128, 128], F32, name="pv", tag="pss")
                    for kc in range(2):
                        nc.tensor.matmul(
                            out=pv[:],
                            lhsT=wv2e_sb[:, kc * D + m * 128:kc * D + (m + 1) * 128],
                            rhs=e_meanT[:, kc * 128:(kc + 1) * 128],
                            start=(kc == 0), stop=(kc == 1),
                        )
                    nc.scalar.activation(
                        out=e_tmpT[:, m * 128:(m + 1) * 128], in_=pv[:],
                        func=mybir.ActivationFunctionType.Relu,
                        bias=bv2e_sb[:, m:m + 1],
                    )

                # embE_newT = W_fuse-transform([embET; e_tmpT]) + b_fuse  [d', seg]
                enT = wp.tile([128, D], F32, name="enT")
                for m in range(2):
                    pf = pss.tile([128, 128], F32, name="pf", tag="pss")
                    for kc in range(4):
                        if kc < 2:
                            rhs = embET_sb[:, kc * W1COLS + w * 128:
                                           kc * W1COLS + (w + 1) * 128]
                        else:
                            rhs = e_tmpT[:, (kc - 2) * 128:(kc - 1) * 128]
                        nc.tensor.matmul(
                            out=pf[:],
                            lhsT=wfuse_sb[:, kc * D + m * 128:kc * D + (m + 1) * 128],
                            rhs=rhs,
                            start=(kc == 0), stop=(kc == 3),
                        )
                    nc.scalar.activation(
                        out=enT[:, m * 128:(m + 1) * 128], in_=pf[:],
                        func=mybir.ActivationFunctionType.Identity,
                        bias=bfuse_sb[:, m:m + 1],
                    )

                # g = embE_new @ W_e2v (+ b_e2v)  [seg, d']  (row-major)
                ps_g = psb.tile([128, D], F32, name="ps_g", tag="psb")
                for kc in range(2):
                    nc.tensor.matmul(
                        out=ps_g[:],
                        lhsT=enT[:, kc * 128:(kc + 1) * 128],
                        rhs=we2v_sb[:, kc * D:(kc + 1) * D],
                        start=(kc == 0), stop=(kc == 1),
                    )
                gf = wp.tile([128, D], F32, name="gf")
                nc.vector.tensor_tensor(out=gf[:], in0=ps_g[:], in1=be2v_sb[:],
                                        op=mybir.AluOpType.add)
                # bf16 hi/lo split, packed [hi | lo]
                ghilo = wp.tile([128, 2 * D], BF16, name="ghilo")
                nc.scalar.copy(ghilo[:, :D], gf[:])
                ghi32 = wp.tile([128, D], F32, name="ghi32")
                nc.scalar.copy(ghi32[:], ghilo[:, :D])
                nc.vector.tensor_tensor(out=ghilo[:, D:], in0=gf[:], in1=ghi32[:],
                                        op=mybir.AluOpType.subtract)
                nc.sync.dma_start(g_loc[w * 128:w * 128 + nseg, :], ghilo[:nseg, :])

                # emb_E_new output (un-transpose enT)
                en = wp.tile([128, D], F32, name="en")
                for c in range(2):
                    pt2 = pss.tile([128, 128], F32, name="pt_en", tag="pss")
                    nc.tensor.transpose(pt2[:], enT[:, c * 128:(c + 1) * 128],
                                        ident_sb[:])
                    nc.scalar.copy(en[:, c * 128:(c + 1) * 128], pt2[:])
                nc.sync.dma_start(embE_new[w * 128:w * 128 + nseg, :], en[:nseg, :])

            # ================= AllGather g =================
            if do_cc:
                nc.gpsimd.collective_compute(
                "AllGather",
                mybir.AluOpType.bypass,
                    replica_groups=[list(range(NCORES))],
                    ins=[g_loc.opt()],
                    outs=[g_full.opt()],
                )

            # ================= HOP 2 =================
            for call in range(NW2 // SW2 if do_hop2 else 0):
                nidx = SW2 * nb2 * 128
                ghl2 = g2p.tile([128, SW2 * nb2, 2 * D], BF16, name="ghl2")
                nc.gpsimd.dma_gather(
                    out_ap=ghl2[:],
                    in_ap=g_full[:, :],
                    idxs_ap=idx2_sb[:, call * SW2 * nb2 * 8:
                                    (call + 1) * SW2 * nb2 * 8],
                    num_idxs=nidx,
                    num_idxs_reg=nidx,
                    elem_size=2 * D,
                    single_packet=False,
                )
                for j in range(SW2):
                    w = call * SW2 + j
                    nnode = NV_SH - w * 128 if w == NW2 - 1 else 128

                    ind2 = indp.tile([128, nb2, 128], BF16, name="ind2", tag="ind")
                    nc.vector.tensor_tensor(
                        out=ind2[:],
                        in0=dstloc2_sb[:, w * nb2:(w + 1) * nb2]
                            .unsqueeze(2).broadcast_to([128, nb2, 128]),
                        in1=iota_sb[:].unsqueeze(1).broadcast_to([128, nb2, 128]),
                        op=mybir.AluOpType.is_equal,
                    )

                    ps2 = psb.tile([128, D], F32, name="ps2", tag="psb")
                    n_mm = 2 * nb2
                    mm = 0
                    for b in range(nb2):
                        for half in range(2):
                            nc.tensor.matmul(
                                out=ps2[:],
                                lhsT=ind2[:, b, :],
                                rhs=ghl2[:, j * nb2 + b, half * D:(half + 1) * D],
                                start=(mm == 0),
                                stop=(mm == n_mm - 1),
                            )
                            mm += 1

                    vout = wp.tile([128, D], F32, name="vout")
                    nc.scalar.activation(
                        out=vout[:], in_=ps2[:],
                        func=mybir.ActivationFunctionType.Relu,
                        scale=invc2_sb[:, w:w + 1],
                    )
                    nc.sync.dma_start(embV_new[w * 128:w * 128 + nnode, :],
                                      vout[:nnode, :])

    nc.compile()
    return nc


_CACHE = {}


def kernel(emb_V, emb_E, edge_index, W_v2e, b_v2e, W_fuse, b_fuse, W_e2v, b_e2v):
    emb_V = np.asarray(emb_V, np.float32)
    emb_E = np.asarray(emb_E, np.float32)
    edge_index = np.asarray(edge_index)
    src = edge_index[0].astype(np.int64)
    dst = edge_index[1].astype(np.int64)

    inv_e = 1.0 / np.maximum(np.bincount(dst, minlength=NE), 1.0).astype(np.float32)
    inv_v = 1.0 / np.maximum(np.bincount(src, minlength=NV), 1.0).astype(np.float32)

    # ---- per-core graph partitioning; uniform block counts ----
    core_of_dst = dst // NE_SH
    core_of_src = src // NV_SH
    nb1 = nb2 = 1
    pre1, pre2 = [], []
    for k in range(NCORES):
        m1 = core_of_dst == k
        s1, d1 = src[m1], dst[m1] - k * NE_SH
        u, linv = np.unique(s1, return_inverse=True)
        assert len(u) <= LOCAL_TBL, f"core {k}: {len(u)} unique srcs"
        pre1.append((u, linv.astype(np.int64), d1))
        c1 = np.bincount(d1 // 128, minlength=NW1)
        nb1 = max(nb1, int(np.ceil(c1.max() / 128)))

        m2 = core_of_src == k
        s2, d2 = src[m2] - k * NV_SH, dst[m2]
        pre2.append((s2, d2))
        c2 = np.bincount(s2 // 128, minlength=NW2)
        nb2 = max(nb2, int(np.ceil(c2.max() / 128)))

    key = (nb1, nb2)
    if key not in _CACHE:
        _CACHE[key] = build_kernel(nb1, nb2)
    nc = _CACHE[key]

    # ---- shared constant inputs ----
    iota = np.broadcast_to(np.arange(128, dtype=np.float32), (128, 128)).astype(BF)
    ident = np.eye(128, dtype=np.float32)
    bv2e_t = np.asarray(b_v2e, np.float32).reshape(2, 128).T.copy()
    bfuse_t = np.asarray(b_fuse, np.float32).reshape(2, 128).T.copy()
    be2v_b = np.broadcast_to(np.asarray(b_e2v, np.float32), (128, D)).copy()

    in_maps = []
    for k in range(NCORES):
        u, linv, d1 = pre1[k]
        gidx1, dloc1 = _shard_edges(d1, linv, NW1, nb1)
        s2, d2 = pre2[k]
        gidx2, dloc2 = _shard_edges(s2, d2, NW2, nb2)

        hi, lo = _hilo(emb_V[u])
        emb_loc = np.zeros((LOCAL_TBL, 2 * D), BF)
        emb_loc[:len(u), :D] = hi
        emb_loc[:len(u), D:] = lo

        embET = np.zeros((2 * 128, W1COLS), np.float32)
        embET[:, :NE_SH] = emb_E[k * NE_SH:(k + 1) * NE_SH].T

        idx1_arr = np.concatenate(
            [_pack_idx(gidx1[w]) for w in range(NW1)], axis=1)
        idx2_arr = np.concatenate(
            [_pack_idx(gidx2[c * SW2:(c + 1) * SW2].reshape(-1))
             for c in range(NW2 // SW2)], axis=1)

        dstloc1 = np.ascontiguousarray(dloc1.reshape(NW1 * nb1, 128).T).astype(BF)
        dstloc2 = np.ascontiguousarray(dloc2.reshape(NW2 * nb2, 128).T).astype(BF)

        invc1_full = np.ones(NW1 * 128, np.float32)
        invc1_full[:NE_SH] = inv_e[k * NE_SH:(k + 1) * NE_SH]
        invc1 = np.ascontiguousarray(invc1_full.reshape(NW1, 128).T)

        invc2_full = np.ones(NW2 * 128, np.float32)
        invc2_full[:NV_SH] = inv_v[k * NV_SH:(k + 1) * NV_SH]
        invc2 = np.ascontiguousarray(invc2_full.reshape(NW2, 128).T)

        in_maps.append(dict(
            emb_loc=emb_loc, embET=embET,
            wv2e=np.asarray(W_v2e, np.float32),
            wfuse=np.asarray(W_fuse, np.float32),
            we2v=np.asarray(W_e2v, np.float32),
            bv2e=bv2e_t, bfuse=bfuse_t, be2v=be2v_b,
            iota_in=iota, ident_in=ident,
            idx1=idx1_arr, dstloc1=dstloc1, invc1=invc1,
            idx2=idx2_arr, dstloc2=dstloc2, invc2=invc2,
        ))

    res = run_bass_kernel_spmd(nc, in_maps, core_ids=list(range(NCORES)))
    emb_V_new = np.concatenate([res.results[k]["embV_new"] for k in range(NCORES)])
    emb_E_new = np.concatenate([res.results[k]["embE_new"] for k in range(NCORES)])
    return emb_V_new, emb_E_new
